# revision 32
# baseline (speedup 1.0000x reference)
"""GATConv (nn_GATConv_45595372814934) Trainium2 Bass kernel, 8 NeuronCores.

kernel(**inputs) -> [100000, 1, 64] float32.

Strategy (graph/edge parallelism):
- Node/edge shard: core c owns nodes [12500c, 12500(c+1)) and their 16
  out-edges each (src is repeat(arange(N), 16), so edges are contiguous).
- Phase 1 (per core): support shard = x_c @ W' where W' = [W | W@a_dst |
  W@a_src], fp16 rows [support(64) | s_dst | s_src] written into a
  paired-padded layout (2 rows per 512B block) -> AllGather (4 chunks,
  overlapping phase 1) into a full [50176 pairs, 512B] fp16 table in HBM.
- Phase 2 (per core): one batched dma_gather (SWDGE extended inst) per
  PAIR of 128-node supertiles fetches 4096 edge pair-blocks (264B each)
  by int16 pair index (biased by -17408; each node's 16 edges are slot-
  ordered by ascending pair id so the snake's last position holds a
  max-q index, defeating the ucode's trailing-negative trim).  Per-edge
  weights are computed for BOTH rows of each gathered pair; the wrong-
  parity half gets -40 added to its leaky-relu score before exp, so its
  weight underflows to zero.  Weighted sum over the 32 half-slots is a
  single fp16 multiply + 5 halving adds (no matmuls/PSUM in phase 2).
  Bottleneck: SWDGE Q7 descriptor generation (~5 ns/descriptor, one
  descriptor per edge, engine-serialized) ~= 1.0 ms/core.
"""

import os
import sys

sys.path.insert(0, "/opt/trn_rl_repo")

import numpy as np

import concourse.bacc as bacc
import concourse.bass as bass
import concourse.mybir as mybir
import concourse.tile as tile
from concourse import bass_utils, library_config
from concourse.bass import AP

F32 = mybir.dt.float32
F16 = mybir.dt.float16
I16 = mybir.dt.int16

N_NODES = 100000
IN_CH = 256
C = 64
DEG = 16
NEG_SLOPE = 0.2
NCORES = 8
NPC = N_NODES // NCORES          # 12500 real nodes per core
NPAD = -(-NPC // 128) * 128      # 12544
SUP = NPAD // 128                # 98 super-tiles
TW = 66                          # table row: support(64) | s_dst | s_src
PB = 256                         # fp16 elements per pair block (512 B)
ELEM = 2 * TW                    # gathered elements per edge (264 B)
NPAIRS_C = NPAD // 2             # 6272 pairs per core shard
NPAIRS = NCORES * NPAIRS_C       # 50176 global pairs
BIAS = 17408                     # idx16 = pair - BIAS (range [-17408, 32767])
NIDX = 2 * DEG * 64              # 2048 edges, no sentinel: per-node ascending-q
                                 # order puts a (w.h.p. non-negative) max-q idx
                                 # at position 2047, defeating trailing trim
JCOL = 16                        # output columns
ICOL = 128                       # snake cols: 2048/16
NCHUNK = 4                       # AllGather chunks (overlap with phase 1)
CH_SUP = [0, 30, 56, 82, 98]     # last chunk small: shorter serial AG tail

LAST_EXEC_NS = None
_CACHED_NC = None


def _mkap(base: AP, extra_off: int, dims) -> AP:
    return AP(base.tensor, base.offset + extra_off,
              [list(base.ap[0])] + [list(d) for d in dims])


def _dma_gather_raw(nc, out_ap, in_ap, idxs_ap, num_idxs_acc, num_idxs,
                    elem_size, elem_step, queue_num):
    """bass.dma_gather minus the elem_size%256 assert (ucode takes any u16)."""
    g = nc.gpsimd
    stride_bytes = elem_step * mybir.dt.size(in_ap.dtype)
    stride_bytes_256 = stride_bytes // 256
    assert stride_bytes % 256 == 0 and stride_bytes_256 < 256
    _in_ap = g.lower_ap_dma(in_ap, for_custom_bir_dma=True)
    _idxs_ap = g.lower_ap(idxs_ap)
    _out_ap = g.lower_ap(out_ap)
    return g.add_instruction(
        mybir.InstDMAGatherAnt(
            name=nc.get_next_instruction_name(),
            ins=[*_in_ap, _idxs_ap, num_idxs_acc],
            outs=[_out_ap],
            transpose=False,
            num_idxs=num_idxs,
            elem_size=elem_size,
            stride_bytes_256=stride_bytes_256,
            gen_mode=0,
            single_packet=False,
            queue_num=queue_num,
            sbuf_tokens_per_rank=0,
            sbuf_free_dim_per_rank=0,
            sbuf_free_dim_pad_per_rank=0,
            sbuf_byte_offset=0,
        ))


def _build_nc():
    nc = bacc.Bacc("TRN2", target_bir_lowering=False, debug=False,
                   num_devices=NCORES, num_swdge_queues=4)

    xT_d = nc.dram_tensor("xT", [IN_CH, NPAD], F16, kind="ExternalInput")
    idx_d = nc.dram_tensor("idx", [128, SUP * ICOL], I16, kind="ExternalInput")
    par_d = nc.dram_tensor("par", [128, SUP * 2 * DEG], F16,
                           kind="ExternalInput")
    nlnd_d = nc.dram_tensor("nlnd", [128, SUP], F32, kind="ExternalInput")
    wp_d = nc.dram_tensor("wp", [IN_CH, TW], F16, kind="ExternalInput")
    out_d = nc.dram_tensor("out", [NPAD, C], F32, kind="ExternalOutput")

    from concourse.replica_groups import maybe_share_collective_output_space
    aspace = maybe_share_collective_output_space(
        "AllGather", [list(range(NCORES))])
    shards = [
        nc.dram_tensor(f"shard{k}", [(CH_SUP[k + 1] - CH_SUP[k]) * 64, PB],
                       F16, kind="Internal")
        for k in range(NCHUNK)
    ]
    table_d = nc.dram_tensor("table", [NPAIRS, PB], F16, kind="Internal",
                             addr_space=aspace)

    idx_sb = nc.alloc_sbuf_tensor("idx_sb", [128, SUP * ICOL], I16)
    par_sb = nc.alloc_sbuf_tensor("par_sb", [128, SUP * 2 * DEG], F16)
    ssrc_sb = nc.alloc_sbuf_tensor("ssrc_sb", [128, SUP], F32)
    nlnd_sb = nc.alloc_sbuf_tensor("nlnd_sb", [128, SUP], F32)
    wp_sb = nc.alloc_sbuf_tensor("wp_sb", [128, 2 * TW], F16)

    with tile.TileContext(nc) as tc:
        with (
            tc.tile_pool(name="xp", bufs=4) as xp,
            tc.tile_pool(name="stp", bufs=3) as stp,
            tc.tile_pool(name="gp", bufs=14) as gp,
            tc.tile_pool(name="sp", bufs=4) as sp,
            tc.tile_pool(name="pp", bufs=3) as pp,
            tc.tile_pool(name="hp", bufs=3) as hp,
            tc.tile_pool(name="obp", bufs=4) as obp,
            tc.tile_pool(name="wtp", bufs=2) as wtp,
            tc.tile_pool(name="ps1", bufs=4, space="PSUM") as ps1,
        ):
            nc.gpsimd.load_library(library_config.mlp)
            # bulk index/mask loads ride the (otherwise idle) SWDGE queue so
            # they don't delay the phase-1 xT loads on the sync queue
            nc.sync.dma_start(idx_sb.ap(), idx_d.ap())
            nc.sync.dma_start(par_sb.ap(), par_d.ap())
            nc.sync.dma_start(nlnd_sb.ap(), nlnd_d.ap())
            nc.sync.dma_start(
                wp_sb.ap(), wp_d.ap().rearrange("(a p) c -> p a c", p=128))
            wp3 = wp_sb.ap().rearrange("p (a c) -> p a c", c=TW)
            nreg = nc.gpsimd.lower_val_access(nc.gpsimd.to_reg(NIDX))

            # phase 1: support table shard in paired-padded layout; each
            # chunk's shard is AllGathered as soon as its supertiles finish,
            # overlapping the collective with the rest of phase 1.
            xT3 = xT_d.ap().rearrange("(a p) n -> p a n", p=128)
            tb_off = 0
            for k in range(NCHUNK):
                s0, s1 = CH_SUP[k], CH_SUP[k + 1]
                for s in range(s0, s1, 2):
                    xt2 = xp.tile([128, 2, 256], F16, tag="xt")
                    nc.sync.dma_start(xt2[:],
                                      xT3[:, :, 128 * s:128 * (s + 2)])
                    ps = ps1.tile([128, 2, TW], F32, tag="ps1")
                    for h in range(2):
                        nc.tensor.matmul(
                            out=ps[:, h, :], lhsT=xt2[:, 0, 128 * h:128 * h + 128],
                            rhs=wp3[:, 0, :], start=True, stop=False)
                        nc.tensor.matmul(
                            out=ps[:, h, :], lhsT=xt2[:, 1, 128 * h:128 * h + 128],
                            rhs=wp3[:, 1, :], start=False, stop=True)
                    st = stp.tile([128, 2, TW], F16, tag="st")
                    nc.scalar.copy(st[:], ps[:])
                    nc.vector.tensor_copy(ssrc_sb.ap()[:, s:s + 2],
                                          _mkap(ps[:], TW - 1, [[TW, 2]]))
                    for h in range(2):
                        dst = AP(shards[k].ap().tensor,
                                 (s - s0 + h) * 64 * PB,
                                 [[PB, 64], [TW, 2], [1, TW]])
                        nc.scalar.dma_start(dst, st[:, h, :])
                # table layout [chunk, core, pairs]: contiguous output slice
                npk = NCORES * (s1 - s0) * 64
                nc.gpsimd.collective_compute(
                    "AllGather", mybir.AluOpType.bypass,
                    replica_groups=[list(range(NCORES))],
                    ins=[shards[k].ap()],
                    outs=[table_d.ap()[tb_off:tb_off + npk, :]])
                tb_off += npk

            # phase 2: batched pair-gather (2 supertiles per instruction;
            # uCode allows up to 4096 indices) + parity-masked weights +
            # weighted halving reduce
            out3 = out_d.ap().rearrange("(s p) c -> p s c", p=128)
            tb_ap = table_d.ap()[BIAS:, :ELEM]
            for s in range(SUP):
                if s % 2 == 0:
                    G2 = gp.tile([128, 2 * JCOL, ELEM], F16, tag="G")
                    _dma_gather_raw(
                        nc, G2[:], tb_ap,
                        idx_sb.ap()[:, ICOL * s:ICOL * (s + 2)],
                        nreg2, 2 * NIDX, ELEM, PB, queue_num=(s // 2) % 4)
                goff = (s % 2) * JCOL * ELEM

                # scores for both halves of each pair: z = s_dst + s_src
                g_sd = _mkap(G2[:], goff + C, [[TW, 2 * DEG]])
                z2 = sp.tile([128, 2 * DEG], F32, tag="z2")
                nc.vector.tensor_scalar(
                    out=z2[:], in0=g_sd,
                    scalar1=ssrc_sb.ap()[:, s:s + 1], scalar2=None,
                    op0=mybir.AluOpType.add)
                lr = sp.tile([128, 2 * DEG], F32, tag="lr")
                nc.vector.scalar_tensor_tensor(
                    out=lr[:], in0=z2[:], scalar=NEG_SLOPE, in1=z2[:],
                    op0=mybir.AluOpType.mult, op1=mybir.AluOpType.max)
                # wrong-parity half gets -40 => exp underflows to 0
                lrm = sp.tile([128, 2 * DEG], F32, tag="lrm")
                nc.vector.tensor_tensor(
                    out=lrm[:], in0=lr[:],
                    in1=par_sb.ap()[:, 2 * DEG * s:2 * DEG * (s + 1)],
                    op=mybir.AluOpType.add)
                wt = sp.tile([128, 2 * DEG], F16, tag="wt")
                nc.scalar.activation(
                    wt[:], lrm[:], mybir.ActivationFunctionType.Exp,
                    bias=nlnd_sb.ap()[:, s:s + 1])

                g_sup = _mkap(G[:], 0, [[TW, 2 * DEG], [1, C]])
                prod = pp.tile([128, 2 * DEG, C], F16, tag="prod")
                nc.vector.tensor_tensor(
                    out=prod[:], in0=g_sup,
                    in1=wt[:].to_broadcast([128, 2 * DEG, C]),
                    op=mybir.AluOpType.mult)

                h1 = hp.tile([128, DEG, C], F16, tag="h1")
                nc.vector.tensor_tensor(out=h1[:], in0=prod[:, :DEG, :],
                                        in1=prod[:, DEG:, :],
                                        op=mybir.AluOpType.add)
                h2 = hp.tile([128, DEG // 2, C], F16, tag="h2")
                nc.vector.tensor_tensor(out=h2[:], in0=h1[:, :DEG // 2, :],
                                        in1=h1[:, DEG // 2:, :],
                                        op=mybir.AluOpType.add)
                h3 = hp.tile([128, DEG // 4, C], F16, tag="h3")
                nc.vector.tensor_tensor(out=h3[:], in0=h2[:, :DEG // 4, :],
                                        in1=h2[:, DEG // 4:, :],
                                        op=mybir.AluOpType.add)
                h4 = hp.tile([128, 2, C], F16, tag="h4")
                nc.vector.tensor_tensor(out=h4[:], in0=h3[:, :2, :],
                                        in1=h3[:, 2:, :],
                                        op=mybir.AluOpType.add)
                ob = obp.tile([128, C], F32, tag="ob")
                nc.vector.tensor_tensor(out=ob[:], in0=h4[:, 0, :],
                                        in1=h4[:, 1, :],
                                        op=mybir.AluOpType.add)
                nc.scalar.dma_start(out3[:, s:s + 1, :], ob[:])

    nc.compile()
    return nc


def _host_prep(x, dst, adj_values, weight, attention):
    dst = np.asarray(dst)
    core = (dst // NPC).astype(np.int32)
    local = (dst % NPC).astype(np.int32)
    ql = local >> 1                      # local pair within core shard
    # table layout [chunk, core, pairs]: global pair id
    pstart = np.asarray([s * 64 for s in CH_SUP], np.int32)
    chunk = np.searchsorted(pstart[1:], ql, side="right").astype(np.int32)
    size_k = (pstart[1:] - pstart[:-1])[chunk]
    q_all = (NCORES * pstart[chunk] + core * size_k
             + (ql - pstart[chunk])).astype(np.int32)
    p_all = (local & 1).astype(np.int32)

    weight = np.asarray(weight, np.float32)
    att = np.asarray(attention, np.float32).reshape(2 * C)
    a_src, a_dst = att[:C], att[C:]
    wp = np.empty((IN_CH, TW), np.float32)
    wp[:, :C] = weight
    wp[:, C] = weight @ a_dst
    wp[:, C + 1] = weight @ a_src
    wp = np.ascontiguousarray(wp.astype(np.float16))

    adj = np.asarray(adj_values, np.float32).reshape(N_NODES, DEG)
    deg = adj.sum(axis=1)

    in_maps = []
    for c in range(NCORES):
        xT = np.zeros((IN_CH, NPAD), np.float16)
        xT[:, :NPC] = np.asarray(x[c * NPC:(c + 1) * NPC], np.float32).T
        nlnd = np.full((NPAD,), -np.log(np.float32(DEG)), np.float32)
        nlnd[:NPC] = -np.log(deg[c * NPC:(c + 1) * NPC])
        nlnd = np.ascontiguousarray(nlnd.reshape(SUP, 128).T)

        qc = np.full((NPAD, DEG), BIAS, np.int32)
        pc = np.zeros((NPAD, DEG), np.int32)
        sl = slice(c * NPC * DEG, (c + 1) * NPC * DEG)
        qc[:NPC] = q_all[sl].reshape(NPC, DEG)
        pc[:NPC] = p_all[sl].reshape(NPC, DEG)

        # per-node ascending-q slot order (slot 15 = max q): the snake's
        # last position (node 128s+127, slot 15) is then non-negative after
        # bias w.h.p., so the ucode's trailing-negative trim never fires.
        order = np.argsort(qc, axis=1, kind="stable")
        qc = np.take_along_axis(qc, order, axis=1)
        pc = np.take_along_axis(pc, order, axis=1)
        if (qc[127::128, DEG - 1] < BIAS).any():
            return None  # pathological input: caller falls back to numpy

        # idx snake: per supertile s, logical index k=j*128+p -> value
        # qc[128s+p, j]-BIAS at snake position [k%16 (replicated x8), k//16]
        idx_k = (qc.reshape(SUP, 128, DEG).transpose(0, 2, 1)
                   .reshape(SUP, 2 * DEG * 64) - BIAS).astype(np.int16)
        snake = idx_k.reshape(SUP, 128, 16).transpose(0, 2, 1)
        idx16 = np.tile(
            snake.transpose(1, 0, 2).reshape(16, SUP * ICOL), (8, 1))

        # parity shift: 0 where half h matches edge parity, -40 otherwise
        par2 = np.full((SUP, 128, DEG, 2), np.float16(-40.0), np.float16)
        pcs = pc.reshape(SUP, 128, DEG)
        one = np.arange(2)[None, None, None, :] == pcs[..., None]
        par2[one] = np.float16(0.0)
        par = np.ascontiguousarray(
            par2.reshape(SUP, 128, 2 * DEG).transpose(1, 0, 2)
                .reshape(128, SUP * 2 * DEG))

        in_maps.append({
            "xT": xT,
            "idx": np.ascontiguousarray(idx16),
            "par": par,
            "nlnd": nlnd,
            "wp": wp,
        })
    return in_maps


def _numpy_fallback(x, edge_index, adj_values, weight, attention):
    N = x.shape[0]
    x = np.asarray(x, np.float32)
    support = (x @ np.asarray(weight, np.float32)).reshape(N, 1, C)
    src = np.asarray(edge_index[0])
    dst = np.asarray(edge_index[1])
    att = np.asarray(attention, np.float32).reshape(1, 1, 2 * C)
    a_src, a_dst = att[0, :, :C], att[0, :, C:]
    s_src = np.einsum('nhc,hc->nh', support, a_src)
    s_dst = np.einsum('nhc,hc->nh', support, a_dst)
    z = s_src[src] + s_dst[dst]
    edge_e = np.exp(np.where(z >= 0, z, NEG_SLOPE * z))
    deg = np.zeros(N, np.float32)
    np.add.at(deg, src, np.asarray(adj_values, np.float32))
    edge_e = edge_e / deg[src][:, None]
    out = np.zeros((N, 1, C), np.float32)
    np.add.at(out, src, edge_e[:, :, None] * support[dst])
    return out.astype(np.float32)


def kernel(x, edge_index, adj_values, weight, attention):
    global LAST_EXEC_NS, _CACHED_NC
    x = np.asarray(x)
    edge_index = np.asarray(edge_index)
    src = edge_index[0]

    expected_src = np.repeat(
        np.arange(N_NODES, dtype=src.dtype), DEG)
    if x.shape[0] != N_NODES or not np.array_equal(src, expected_src):
        # unexpected structure: fall back to a host reference implementation
        return _numpy_fallback(x, edge_index, adj_values, weight, attention)

    if _CACHED_NC is None:
        _CACHED_NC = _build_nc()
    nc = _CACHED_NC

    in_maps = _host_prep(x, edge_index[1], adj_values, weight, attention)
    if in_maps is None:
        return _numpy_fallback(x, edge_index, adj_values, weight, attention)

    trace = os.environ.get("GAT_BASS_TRACE", "") == "1"
    kwargs = {}
    if trace:
        try:
            import prof_hook
            prof_hook.install()
        except Exception:
            trace = False
    res = bass_utils.run_bass_kernel_spmd(
        nc, in_maps, core_ids=list(range(NCORES)), trace=trace)
    LAST_EXEC_NS = res.exec_time_ns

    parts = [res.results[c]["out"][:NPC] for c in range(NCORES)]
    out = np.concatenate(parts, 0).reshape(N_NODES, 1, C)
    return np.ascontiguousarray(out.astype(np.float32))


# revision 34
# speedup vs baseline: 1.5695x; 1.5695x over previous
"""GATConv (nn_GATConv_45595372814934) Trainium2 Bass kernel, 8 NeuronCores.

kernel(**inputs) -> [100000, 1, 64] float32.

Strategy (graph/edge parallelism):
- Node/edge shard: core c owns nodes [12500c, 12500(c+1)) and their 16
  out-edges each (src is repeat(arange(N), 16), so edges are contiguous).
- Phase 1 (per core): support shard = x_c @ W' where W' = [W | W@a_dst |
  W@a_src], fp16 rows [support(64) | s_dst | s_src] written into a
  paired-padded layout (2 rows per 512B block) -> AllGather (4 chunks,
  overlapping phase 1) into a full [50176 pairs, 512B] fp16 table in HBM.
- Phase 2 (per core): one batched dma_gather (SWDGE extended inst) per
  PAIR of 128-node supertiles fetches 4096 edge pair-blocks (264B each)
  by int16 pair index (biased by -17408; each node's 16 edges are slot-
  ordered by ascending pair id so the snake's last position holds a
  max-q index, defeating the ucode's trailing-negative trim).  Per-edge
  weights are computed for BOTH rows of each gathered pair; the wrong-
  parity half gets -40 added to its leaky-relu score before exp, so its
  weight underflows to zero.  Weighted sum over the 32 half-slots is a
  single fp16 multiply + 5 halving adds (no matmuls/PSUM in phase 2).
  Bottleneck: SWDGE Q7 descriptor generation (~5 ns/descriptor, one
  descriptor per edge, engine-serialized) ~= 1.0 ms/core.
"""

import os
import sys

sys.path.insert(0, "/opt/trn_rl_repo")

import numpy as np

import concourse.bacc as bacc
import concourse.bass as bass
import concourse.mybir as mybir
import concourse.tile as tile
from concourse import bass_utils, library_config
from concourse.bass import AP

F32 = mybir.dt.float32
F16 = mybir.dt.float16
I16 = mybir.dt.int16

N_NODES = 100000
IN_CH = 256
C = 64
DEG = 16
NEG_SLOPE = 0.2
NCORES = 8
NPC = N_NODES // NCORES          # 12500 real nodes per core
NPAD = -(-NPC // 128) * 128      # 12544
SUP = NPAD // 128                # 98 super-tiles
TW = 66                          # table row: support(64) | s_dst | s_src
PB = 256                         # fp16 elements per pair block (512 B)
ELEM = 2 * TW                    # gathered elements per edge (264 B)
NPAIRS_C = NPAD // 2             # 6272 pairs per core shard
NPAIRS = NCORES * NPAIRS_C       # 50176 global pairs
BIAS = 17408                     # idx16 = pair - BIAS (range [-17408, 32767])
NIDX = 2 * DEG * 64              # 2048 edges, no sentinel: per-node ascending-q
                                 # order puts a (w.h.p. non-negative) max-q idx
                                 # at position 2047, defeating trailing trim
JCOL = 16                        # output columns
ICOL = 128                       # snake cols: 2048/16
NCHUNK = 4                       # AllGather chunks (overlap with phase 1)
CH_SUP = [0, 30, 56, 82, 98]     # last chunk small: shorter serial AG tail

LAST_EXEC_NS = None
_CACHED_NC = None


def _mkap(base: AP, extra_off: int, dims) -> AP:
    return AP(base.tensor, base.offset + extra_off,
              [list(base.ap[0])] + [list(d) for d in dims])


def _dma_gather_raw(nc, out_ap, in_ap, idxs_ap, num_idxs_acc, num_idxs,
                    elem_size, elem_step, queue_num):
    """bass.dma_gather minus the elem_size%256 assert (ucode takes any u16)."""
    g = nc.gpsimd
    stride_bytes = elem_step * mybir.dt.size(in_ap.dtype)
    stride_bytes_256 = stride_bytes // 256
    assert stride_bytes % 256 == 0 and stride_bytes_256 < 256
    _in_ap = g.lower_ap_dma(in_ap, for_custom_bir_dma=True)
    _idxs_ap = g.lower_ap(idxs_ap)
    _out_ap = g.lower_ap(out_ap)
    return g.add_instruction(
        mybir.InstDMAGatherAnt(
            name=nc.get_next_instruction_name(),
            ins=[*_in_ap, _idxs_ap, num_idxs_acc],
            outs=[_out_ap],
            transpose=False,
            num_idxs=num_idxs,
            elem_size=elem_size,
            stride_bytes_256=stride_bytes_256,
            gen_mode=0,
            single_packet=False,
            queue_num=queue_num,
            sbuf_tokens_per_rank=0,
            sbuf_free_dim_per_rank=0,
            sbuf_free_dim_pad_per_rank=0,
            sbuf_byte_offset=0,
        ))


def _build_nc():
    nc = bacc.Bacc("TRN2", target_bir_lowering=False, debug=False,
                   num_devices=NCORES, num_swdge_queues=4)

    xT_d = nc.dram_tensor("xT", [IN_CH, NPAD], F16, kind="ExternalInput")
    idx_d = nc.dram_tensor("idx", [128, SUP * ICOL], I16, kind="ExternalInput")
    par_d = nc.dram_tensor("par", [128, SUP * 2 * DEG], F16,
                           kind="ExternalInput")
    nlnd_d = nc.dram_tensor("nlnd", [128, SUP], F32, kind="ExternalInput")
    wp_d = nc.dram_tensor("wp", [IN_CH, TW], F16, kind="ExternalInput")
    out_d = nc.dram_tensor("out", [NPAD, C], F32, kind="ExternalOutput")

    from concourse.replica_groups import maybe_share_collective_output_space
    aspace = maybe_share_collective_output_space(
        "AllGather", [list(range(NCORES))])
    shards = [
        nc.dram_tensor(f"shard{k}", [(CH_SUP[k + 1] - CH_SUP[k]) * 64, PB],
                       F16, kind="Internal")
        for k in range(NCHUNK)
    ]
    table_d = nc.dram_tensor("table", [NPAIRS, PB], F16, kind="Internal",
                             addr_space=aspace)

    idx_sb = nc.alloc_sbuf_tensor("idx_sb", [128, SUP * ICOL], I16)
    par_sb = nc.alloc_sbuf_tensor("par_sb", [128, SUP * 2 * DEG], F16)
    ssrc_sb = nc.alloc_sbuf_tensor("ssrc_sb", [128, SUP], F32)
    nlnd_sb = nc.alloc_sbuf_tensor("nlnd_sb", [128, SUP], F32)
    wp_sb = nc.alloc_sbuf_tensor("wp_sb", [128, 2 * TW], F16)

    with tile.TileContext(nc) as tc:
        with (
            tc.tile_pool(name="xp", bufs=4) as xp,
            tc.tile_pool(name="stp", bufs=3) as stp,
            tc.tile_pool(name="gp", bufs=14) as gp,
            tc.tile_pool(name="sp", bufs=4) as sp,
            tc.tile_pool(name="pp", bufs=4) as pp,
            tc.tile_pool(name="hp", bufs=3) as hp,
            tc.tile_pool(name="obp", bufs=4) as obp,
            tc.tile_pool(name="wtp", bufs=2) as wtp,
            tc.tile_pool(name="ps1", bufs=4, space="PSUM") as ps1,
        ):
            nc.gpsimd.load_library(library_config.mlp)
            # bulk index/mask loads ride the (otherwise idle) SWDGE queue so
            # they don't delay the phase-1 xT loads on the sync queue
            nc.sync.dma_start(idx_sb.ap(), idx_d.ap())
            nc.sync.dma_start(par_sb.ap(), par_d.ap())
            nc.sync.dma_start(nlnd_sb.ap(), nlnd_d.ap())
            nc.sync.dma_start(
                wp_sb.ap(), wp_d.ap().rearrange("(a p) c -> p a c", p=128))
            wp3 = wp_sb.ap().rearrange("p (a c) -> p a c", c=TW)
            nreg = nc.gpsimd.lower_val_access(nc.gpsimd.to_reg(NIDX))

            # phase 1: support table shard in paired-padded layout; each
            # chunk's shard is AllGathered as soon as its supertiles finish,
            # overlapping the collective with the rest of phase 1.
            xT3 = xT_d.ap().rearrange("(a p) n -> p a n", p=128)
            tb_off = 0
            for k in range(NCHUNK):
                s0, s1 = CH_SUP[k], CH_SUP[k + 1]
                for s in range(s0, s1, 4):
                    w = min(4, s1 - s)
                    xt4 = xp.tile([128, 2, 512], F16, tag="xt")
                    nc.sync.dma_start(xt4[:, :, :128 * w],
                                      xT3[:, :, 128 * s:128 * (s + w)])
                    ps = ps1.tile([128, 4, TW], F32, tag="ps1")
                    for h in range(w):
                        nc.tensor.matmul(
                            out=ps[:, h, :], lhsT=xt4[:, 0, 128 * h:128 * h + 128],
                            rhs=wp3[:, 0, :], start=True, stop=False)
                        nc.tensor.matmul(
                            out=ps[:, h, :], lhsT=xt4[:, 1, 128 * h:128 * h + 128],
                            rhs=wp3[:, 1, :], start=False, stop=True)
                    st = stp.tile([128, 4, TW], F16, tag="st")
                    nc.scalar.copy(st[:, :w, :], ps[:, :w, :])
                    nc.vector.tensor_copy(ssrc_sb.ap()[:, s:s + w],
                                          _mkap(ps[:], TW - 1, [[TW, w]]))
                    for h in range(w):
                        dst = AP(shards[k].ap().tensor,
                                 (s - s0 + h) * 64 * PB,
                                 [[PB, 64], [TW, 2], [1, TW]])
                        nc.scalar.dma_start(dst, st[:, h, :])
                # table layout [chunk, core, pairs]: contiguous output slice
                npk = NCORES * (s1 - s0) * 64
                nc.gpsimd.collective_compute(
                    "AllGather", mybir.AluOpType.bypass,
                    replica_groups=[list(range(NCORES))],
                    ins=[shards[k].ap()],
                    outs=[table_d.ap()[tb_off:tb_off + npk, :]])
                tb_off += npk

            # phase 2: batched pair-gather (2 supertiles per instruction;
            # uCode allows up to 4096 indices) + parity-masked weights +
            # weighted halving reduce
            out3 = out_d.ap().rearrange("(s p) c -> p s c", p=128)
            tb_ap = table_d.ap()[BIAS:, :ELEM]
            for s in range(SUP):
                if s % 2 == 0:
                    G2 = gp.tile([128, 2 * JCOL, ELEM], F16, tag="G")
                    _dma_gather_raw(
                        nc, G2[:], tb_ap,
                        idx_sb.ap()[:, ICOL * s:ICOL * (s + 2)],
                        nreg2, 2 * NIDX, ELEM, PB, queue_num=(s // 2) % 4)
                goff = (s % 2) * JCOL * ELEM

                # scores for both halves of each pair: z = s_dst + s_src
                g_sd = _mkap(G2[:], goff + C, [[TW, 2 * DEG]])
                z2 = sp.tile([128, 2 * DEG], F32, tag="z2")
                nc.vector.tensor_scalar(
                    out=z2[:], in0=g_sd,
                    scalar1=ssrc_sb.ap()[:, s:s + 1], scalar2=None,
                    op0=mybir.AluOpType.add)
                lr = sp.tile([128, 2 * DEG], F32, tag="lr")
                nc.vector.scalar_tensor_tensor(
                    out=lr[:], in0=z2[:], scalar=NEG_SLOPE, in1=z2[:],
                    op0=mybir.AluOpType.mult, op1=mybir.AluOpType.max)
                # wrong-parity half gets -40 => exp underflows to 0
                lrm = sp.tile([128, 2 * DEG], F32, tag="lrm")
                nc.vector.tensor_tensor(
                    out=lrm[:], in0=lr[:],
                    in1=par_sb.ap()[:, 2 * DEG * s:2 * DEG * (s + 1)],
                    op=mybir.AluOpType.add)
                wt = sp.tile([128, 2 * DEG], F16, tag="wt")
                nc.scalar.activation(
                    wt[:], lrm[:], mybir.ActivationFunctionType.Exp,
                    bias=nlnd_sb.ap()[:, s:s + 1])

                g_sup = _mkap(G[:], 0, [[TW, 2 * DEG], [1, C]])
                prod = pp.tile([128, 2 * DEG, C], F16, tag="prod")
                nc.vector.tensor_tensor(
                    out=prod[:], in0=g_sup,
                    in1=wt[:].to_broadcast([128, 2 * DEG, C]),
                    op=mybir.AluOpType.mult)

                h1 = hp.tile([128, DEG, C], F16, tag="h1")
                nc.vector.tensor_tensor(out=h1[:], in0=prod[:, :DEG, :],
                                        in1=prod[:, DEG:, :],
                                        op=mybir.AluOpType.add)
                h2 = hp.tile([128, DEG // 2, C], F16, tag="h2")
                nc.vector.tensor_tensor(out=h2[:], in0=h1[:, :DEG // 2, :],
                                        in1=h1[:, DEG // 2:, :],
                                        op=mybir.AluOpType.add)
                h3 = hp.tile([128, DEG // 4, C], F16, tag="h3")
                nc.vector.tensor_tensor(out=h3[:], in0=h2[:, :DEG // 4, :],
                                        in1=h2[:, DEG // 4:, :],
                                        op=mybir.AluOpType.add)
                h4 = hp.tile([128, 2, C], F16, tag="h4")
                nc.vector.tensor_tensor(out=h4[:], in0=h3[:, :2, :],
                                        in1=h3[:, 2:, :],
                                        op=mybir.AluOpType.add)
                ob = obp.tile([128, C], F32, tag="ob")
                nc.vector.tensor_tensor(out=ob[:], in0=h4[:, 0, :],
                                        in1=h4[:, 1, :],
                                        op=mybir.AluOpType.add)
                nc.scalar.dma_start(out3[:, s:s + 1, :], ob[:])

    nc.compile()
    return nc


def _host_prep(x, dst, adj_values, weight, attention):
    dst = np.asarray(dst)
    core = (dst // NPC).astype(np.int32)
    local = (dst % NPC).astype(np.int32)
    ql = local >> 1                      # local pair within core shard
    # table layout [chunk, core, pairs]: global pair id
    pstart = np.asarray([s * 64 for s in CH_SUP], np.int32)
    chunk = np.searchsorted(pstart[1:], ql, side="right").astype(np.int32)
    size_k = (pstart[1:] - pstart[:-1])[chunk]
    q_all = (NCORES * pstart[chunk] + core * size_k
             + (ql - pstart[chunk])).astype(np.int32)
    p_all = (local & 1).astype(np.int32)

    weight = np.asarray(weight, np.float32)
    att = np.asarray(attention, np.float32).reshape(2 * C)
    a_src, a_dst = att[:C], att[C:]
    wp = np.empty((IN_CH, TW), np.float32)
    wp[:, :C] = weight
    wp[:, C] = weight @ a_dst
    wp[:, C + 1] = weight @ a_src
    wp = np.ascontiguousarray(wp.astype(np.float16))

    adj = np.asarray(adj_values, np.float32).reshape(N_NODES, DEG)
    deg = adj.sum(axis=1)

    in_maps = []
    for c in range(NCORES):
        xT = np.zeros((IN_CH, NPAD), np.float16)
        xT[:, :NPC] = np.asarray(x[c * NPC:(c + 1) * NPC], np.float32).T
        nlnd = np.full((NPAD,), -np.log(np.float32(DEG)), np.float32)
        nlnd[:NPC] = -np.log(deg[c * NPC:(c + 1) * NPC])
        nlnd = np.ascontiguousarray(nlnd.reshape(SUP, 128).T)

        qc = np.full((NPAD, DEG), BIAS, np.int32)
        pc = np.zeros((NPAD, DEG), np.int32)
        sl = slice(c * NPC * DEG, (c + 1) * NPC * DEG)
        qc[:NPC] = q_all[sl].reshape(NPC, DEG)
        pc[:NPC] = p_all[sl].reshape(NPC, DEG)

        # per-node ascending-q slot order (slot 15 = max q): the snake's
        # last position (node 128s+127, slot 15) is then non-negative after
        # bias w.h.p., so the ucode's trailing-negative trim never fires.
        order = np.argsort(qc, axis=1, kind="stable")
        qc = np.take_along_axis(qc, order, axis=1)
        pc = np.take_along_axis(pc, order, axis=1)
        if (qc[127::128, DEG - 1] < BIAS).any():
            return None  # pathological input: caller falls back to numpy

        # idx snake: per supertile s, logical index k=j*128+p -> value
        # qc[128s+p, j]-BIAS at snake position [k%16 (replicated x8), k//16]
        idx_k = (qc.reshape(SUP, 128, DEG).transpose(0, 2, 1)
                   .reshape(SUP, 2 * DEG * 64) - BIAS).astype(np.int16)
        snake = idx_k.reshape(SUP, 128, 16).transpose(0, 2, 1)
        idx16 = np.tile(
            snake.transpose(1, 0, 2).reshape(16, SUP * ICOL), (8, 1))

        # parity shift: 0 where half h matches edge parity, -40 otherwise
        par2 = np.full((SUP, 128, DEG, 2), np.float16(-40.0), np.float16)
        pcs = pc.reshape(SUP, 128, DEG)
        one = np.arange(2)[None, None, None, :] == pcs[..., None]
        par2[one] = np.float16(0.0)
        par = np.ascontiguousarray(
            par2.reshape(SUP, 128, 2 * DEG).transpose(1, 0, 2)
                .reshape(128, SUP * 2 * DEG))

        in_maps.append({
            "xT": xT,
            "idx": np.ascontiguousarray(idx16),
            "par": par,
            "nlnd": nlnd,
            "wp": wp,
        })
    return in_maps


def _numpy_fallback(x, edge_index, adj_values, weight, attention):
    N = x.shape[0]
    x = np.asarray(x, np.float32)
    support = (x @ np.asarray(weight, np.float32)).reshape(N, 1, C)
    src = np.asarray(edge_index[0])
    dst = np.asarray(edge_index[1])
    att = np.asarray(attention, np.float32).reshape(1, 1, 2 * C)
    a_src, a_dst = att[0, :, :C], att[0, :, C:]
    s_src = np.einsum('nhc,hc->nh', support, a_src)
    s_dst = np.einsum('nhc,hc->nh', support, a_dst)
    z = s_src[src] + s_dst[dst]
    edge_e = np.exp(np.where(z >= 0, z, NEG_SLOPE * z))
    deg = np.zeros(N, np.float32)
    np.add.at(deg, src, np.asarray(adj_values, np.float32))
    edge_e = edge_e / deg[src][:, None]
    out = np.zeros((N, 1, C), np.float32)
    np.add.at(out, src, edge_e[:, :, None] * support[dst])
    return out.astype(np.float32)


def kernel(x, edge_index, adj_values, weight, attention):
    global LAST_EXEC_NS, _CACHED_NC
    x = np.asarray(x)
    edge_index = np.asarray(edge_index)
    src = edge_index[0]

    expected_src = np.repeat(
        np.arange(N_NODES, dtype=src.dtype), DEG)
    if x.shape[0] != N_NODES or not np.array_equal(src, expected_src):
        # unexpected structure: fall back to a host reference implementation
        return _numpy_fallback(x, edge_index, adj_values, weight, attention)

    if _CACHED_NC is None:
        _CACHED_NC = _build_nc()
    nc = _CACHED_NC

    in_maps = _host_prep(x, edge_index[1], adj_values, weight, attention)
    if in_maps is None:
        return _numpy_fallback(x, edge_index, adj_values, weight, attention)

    trace = os.environ.get("GAT_BASS_TRACE", "") == "1"
    kwargs = {}
    if trace:
        try:
            import prof_hook
            prof_hook.install()
        except Exception:
            trace = False
    res = bass_utils.run_bass_kernel_spmd(
        nc, in_maps, core_ids=list(range(NCORES)), trace=trace)
    LAST_EXEC_NS = res.exec_time_ns

    parts = [res.results[c]["out"][:NPC] for c in range(NCORES)]
    out = np.concatenate(parts, 0).reshape(N_NODES, 1, C)
    return np.ascontiguousarray(out.astype(np.float32))


# revision 35
# speedup vs baseline: 1.6249x; 1.0353x over previous
"""GATConv (nn_GATConv_45595372814934) Trainium2 Bass kernel, 8 NeuronCores.

kernel(**inputs) -> [100000, 1, 64] float32.

Strategy (graph/edge parallelism):
- Node/edge shard: core c owns nodes [12500c, 12500(c+1)) and their 16
  out-edges each (src is repeat(arange(N), 16), so edges are contiguous).
- Phase 1 (per core): support shard = x_c @ W' where W' = [W | W@a_dst |
  W@a_src], fp16 rows [support(64) | s_dst | s_src] written into a
  paired-padded layout (2 rows per 512B block) -> AllGather (4 chunks,
  overlapping phase 1) into a full [50176 pairs, 512B] fp16 table in HBM.
- Phase 2 (per core): one batched dma_gather (SWDGE extended inst) per
  PAIR of 128-node supertiles fetches 4096 edge pair-blocks (264B each)
  by int16 pair index (biased by -17408; each node's 16 edges are slot-
  ordered by ascending pair id so the snake's last position holds a
  max-q index, defeating the ucode's trailing-negative trim).  Per-edge
  weights are computed for BOTH rows of each gathered pair; the wrong-
  parity half gets -40 added to its leaky-relu score before exp, so its
  weight underflows to zero.  Weighted sum over the 32 half-slots is a
  single fp16 multiply + 5 halving adds (no matmuls/PSUM in phase 2).
  Bottleneck: SWDGE Q7 descriptor generation (~5 ns/descriptor, one
  descriptor per edge, engine-serialized) ~= 1.0 ms/core.
"""

import os
import sys

sys.path.insert(0, "/opt/trn_rl_repo")

import numpy as np

import concourse.bacc as bacc
import concourse.bass as bass
import concourse.mybir as mybir
import concourse.tile as tile
from concourse import bass_utils, library_config
from concourse.bass import AP

F32 = mybir.dt.float32
F16 = mybir.dt.float16
I16 = mybir.dt.int16

N_NODES = 100000
IN_CH = 256
C = 64
DEG = 16
NEG_SLOPE = 0.2
NCORES = 8
NPC = N_NODES // NCORES          # 12500 real nodes per core
NPAD = -(-NPC // 128) * 128      # 12544
SUP = NPAD // 128                # 98 super-tiles
TW = 66                          # table row: support(64) | s_dst | s_src
PB = 256                         # fp16 elements per pair block (512 B)
ELEM = 2 * TW                    # gathered elements per edge (264 B)
NPAIRS_C = NPAD // 2             # 6272 pairs per core shard
NPAIRS = NCORES * NPAIRS_C       # 50176 global pairs
BIAS = 17408                     # idx16 = pair - BIAS (range [-17408, 32767])
NIDX = 2 * DEG * 64              # 2048 edges, no sentinel: per-node ascending-q
                                 # order puts a (w.h.p. non-negative) max-q idx
                                 # at position 2047, defeating trailing trim
JCOL = 16                        # output columns
ICOL = 128                       # snake cols: 2048/16
NCHUNK = 4                       # AllGather chunks (overlap with phase 1)
CH_SUP = [0, 30, 56, 82, 98]     # last chunk small: shorter serial AG tail

LAST_EXEC_NS = None
_CACHED_NC = None


def _mkap(base: AP, extra_off: int, dims) -> AP:
    return AP(base.tensor, base.offset + extra_off,
              [list(base.ap[0])] + [list(d) for d in dims])


def _dma_gather_raw(nc, out_ap, in_ap, idxs_ap, num_idxs_acc, num_idxs,
                    elem_size, elem_step, queue_num):
    """bass.dma_gather minus the elem_size%256 assert (ucode takes any u16)."""
    g = nc.gpsimd
    stride_bytes = elem_step * mybir.dt.size(in_ap.dtype)
    stride_bytes_256 = stride_bytes // 256
    assert stride_bytes % 256 == 0 and stride_bytes_256 < 256
    _in_ap = g.lower_ap_dma(in_ap, for_custom_bir_dma=True)
    _idxs_ap = g.lower_ap(idxs_ap)
    _out_ap = g.lower_ap(out_ap)
    return g.add_instruction(
        mybir.InstDMAGatherAnt(
            name=nc.get_next_instruction_name(),
            ins=[*_in_ap, _idxs_ap, num_idxs_acc],
            outs=[_out_ap],
            transpose=False,
            num_idxs=num_idxs,
            elem_size=elem_size,
            stride_bytes_256=stride_bytes_256,
            gen_mode=0,
            single_packet=False,
            queue_num=queue_num,
            sbuf_tokens_per_rank=0,
            sbuf_free_dim_per_rank=0,
            sbuf_free_dim_pad_per_rank=0,
            sbuf_byte_offset=0,
        ))


def _build_nc():
    nc = bacc.Bacc("TRN2", target_bir_lowering=False, debug=False,
                   num_devices=NCORES, num_swdge_queues=4)

    xT_d = nc.dram_tensor("xT", [IN_CH, NPAD], F16, kind="ExternalInput")
    idx_d = nc.dram_tensor("idx", [128, SUP * ICOL], I16, kind="ExternalInput")
    par_d = nc.dram_tensor("par", [128, SUP * 2 * DEG], F16,
                           kind="ExternalInput")
    nlnd_d = nc.dram_tensor("nlnd", [128, SUP], F32, kind="ExternalInput")
    wp_d = nc.dram_tensor("wp", [IN_CH, TW], F16, kind="ExternalInput")
    out_d = nc.dram_tensor("out", [NPAD, C], F32, kind="ExternalOutput")

    from concourse.replica_groups import maybe_share_collective_output_space
    aspace = maybe_share_collective_output_space(
        "AllGather", [list(range(NCORES))])
    shards = [
        nc.dram_tensor(f"shard{k}", [(CH_SUP[k + 1] - CH_SUP[k]) * 64, PB],
                       F16, kind="Internal")
        for k in range(NCHUNK)
    ]
    table_d = nc.dram_tensor("table", [NPAIRS, PB], F16, kind="Internal",
                             addr_space=aspace)

    idx_sb = nc.alloc_sbuf_tensor("idx_sb", [128, SUP * ICOL], I16)
    par_sb = nc.alloc_sbuf_tensor("par_sb", [128, SUP * 2 * DEG], F16)
    ssrc_sb = nc.alloc_sbuf_tensor("ssrc_sb", [128, SUP], F32)
    nlnd_sb = nc.alloc_sbuf_tensor("nlnd_sb", [128, SUP], F32)
    wp_sb = nc.alloc_sbuf_tensor("wp_sb", [128, 2 * TW], F16)

    with tile.TileContext(nc) as tc:
        with (
            tc.tile_pool(name="xp", bufs=4) as xp,
            tc.tile_pool(name="stp", bufs=3) as stp,
            tc.tile_pool(name="gp", bufs=14) as gp,
            tc.tile_pool(name="sp", bufs=6) as sp,
            tc.tile_pool(name="pp", bufs=4) as pp,
            tc.tile_pool(name="hp", bufs=3) as hp,
            tc.tile_pool(name="obp", bufs=4) as obp,
            tc.tile_pool(name="wtp", bufs=4) as wtp,
            tc.tile_pool(name="ps1", bufs=4, space="PSUM") as ps1,
        ):
            nc.gpsimd.load_library(library_config.mlp)
            # bulk index/mask loads ride the (otherwise idle) SWDGE queue so
            # they don't delay the phase-1 xT loads on the sync queue
            nc.sync.dma_start(idx_sb.ap(), idx_d.ap())
            nc.sync.dma_start(par_sb.ap(), par_d.ap())
            nc.sync.dma_start(nlnd_sb.ap(), nlnd_d.ap())
            nc.sync.dma_start(
                wp_sb.ap(), wp_d.ap().rearrange("(a p) c -> p a c", p=128))
            wp3 = wp_sb.ap().rearrange("p (a c) -> p a c", c=TW)
            nreg = nc.gpsimd.lower_val_access(nc.gpsimd.to_reg(NIDX))

            # phase 1: support table shard in paired-padded layout; each
            # chunk's shard is AllGathered as soon as its supertiles finish,
            # overlapping the collective with the rest of phase 1.
            xT3 = xT_d.ap().rearrange("(a p) n -> p a n", p=128)
            tb_off = 0
            for k in range(NCHUNK):
                s0, s1 = CH_SUP[k], CH_SUP[k + 1]
                for s in range(s0, s1, 4):
                    w = min(4, s1 - s)
                    xt4 = xp.tile([128, 2, 512], F16, tag="xt")
                    nc.sync.dma_start(xt4[:, :, :128 * w],
                                      xT3[:, :, 128 * s:128 * (s + w)])
                    ps = ps1.tile([128, 4, TW], F32, tag="ps1")
                    for h in range(w):
                        nc.tensor.matmul(
                            out=ps[:, h, :], lhsT=xt4[:, 0, 128 * h:128 * h + 128],
                            rhs=wp3[:, 0, :], start=True, stop=False)
                        nc.tensor.matmul(
                            out=ps[:, h, :], lhsT=xt4[:, 1, 128 * h:128 * h + 128],
                            rhs=wp3[:, 1, :], start=False, stop=True)
                    st = stp.tile([128, 4, TW], F16, tag="st")
                    nc.scalar.copy(st[:, :w, :], ps[:, :w, :])
                    nc.vector.tensor_copy(ssrc_sb.ap()[:, s:s + w],
                                          _mkap(ps[:], TW - 1, [[TW, w]]))
                    for h in range(w):
                        dst = AP(shards[k].ap().tensor,
                                 (s - s0 + h) * 64 * PB,
                                 [[PB, 64], [TW, 2], [1, TW]])
                        nc.scalar.dma_start(dst, st[:, h, :])
                # table layout [chunk, core, pairs]: contiguous output slice
                npk = NCORES * (s1 - s0) * 64
                nc.gpsimd.collective_compute(
                    "AllGather", mybir.AluOpType.bypass,
                    replica_groups=[list(range(NCORES))],
                    ins=[shards[k].ap()],
                    outs=[table_d.ap()[tb_off:tb_off + npk, :]])
                tb_off += npk

            # phase 2: batched pair-gather (2 supertiles per instruction;
            # uCode allows up to 4096 indices) + parity-masked weights +
            # weighted halving reduce
            out3 = out_d.ap().rearrange("(s p) c -> p s c", p=128)
            tb_ap = table_d.ap()[BIAS:, :ELEM]
            for s in range(SUP):
                if s % 2 == 0:
                    G2 = gp.tile([128, 2 * JCOL, ELEM], F16, tag="G")
                    _dma_gather_raw(
                        nc, G2[:], tb_ap,
                        idx_sb.ap()[:, ICOL * s:ICOL * (s + 2)],
                        nreg2, 2 * NIDX, ELEM, PB, queue_num=(s // 2) % 4)
                goff = (s % 2) * JCOL * ELEM

                # scores for both halves of each pair: z = s_dst + s_src
                g_sd = _mkap(G2[:], goff + C, [[TW, 2 * DEG]])
                z2 = sp.tile([128, 2 * DEG], F32, tag="z2")
                nc.vector.tensor_scalar(
                    out=z2[:], in0=g_sd,
                    scalar1=ssrc_sb.ap()[:, s:s + 1], scalar2=None,
                    op0=mybir.AluOpType.add)
                lr = sp.tile([128, 2 * DEG], F32, tag="lr")
                nc.vector.scalar_tensor_tensor(
                    out=lr[:], in0=z2[:], scalar=NEG_SLOPE, in1=z2[:],
                    op0=mybir.AluOpType.mult, op1=mybir.AluOpType.max)
                # wrong-parity half gets -40 => exp underflows to 0
                lrm = sp.tile([128, 2 * DEG], F32, tag="lrm")
                nc.vector.tensor_tensor(
                    out=lrm[:], in0=lr[:],
                    in1=par_sb.ap()[:, 2 * DEG * s:2 * DEG * (s + 1)],
                    op=mybir.AluOpType.add)
                wt = sp.tile([128, 2 * DEG], F16, tag="wt")
                nc.scalar.activation(
                    wt[:], lrm[:], mybir.ActivationFunctionType.Exp,
                    bias=nlnd_sb.ap()[:, s:s + 1])

                g_sup = _mkap(G[:], 0, [[TW, 2 * DEG], [1, C]])
                prod = pp.tile([128, 2 * DEG, C], F16, tag="prod")
                nc.vector.tensor_tensor(
                    out=prod[:], in0=g_sup,
                    in1=wt[:].to_broadcast([128, 2 * DEG, C]),
                    op=mybir.AluOpType.mult)

                h1 = hp.tile([128, DEG, C], F16, tag="h1")
                nc.vector.tensor_tensor(out=h1[:], in0=prod[:, :DEG, :],
                                        in1=prod[:, DEG:, :],
                                        op=mybir.AluOpType.add)
                h2 = hp.tile([128, DEG // 2, C], F16, tag="h2")
                nc.vector.tensor_tensor(out=h2[:], in0=h1[:, :DEG // 2, :],
                                        in1=h1[:, DEG // 2:, :],
                                        op=mybir.AluOpType.add)
                h3 = hp.tile([128, DEG // 4, C], F16, tag="h3")
                nc.vector.tensor_tensor(out=h3[:], in0=h2[:, :DEG // 4, :],
                                        in1=h2[:, DEG // 4:, :],
                                        op=mybir.AluOpType.add)
                h4 = hp.tile([128, 2, C], F16, tag="h4")
                nc.vector.tensor_tensor(out=h4[:], in0=h3[:, :2, :],
                                        in1=h3[:, 2:, :],
                                        op=mybir.AluOpType.add)
                ob = obp.tile([128, C], F32, tag="ob")
                nc.vector.tensor_tensor(out=ob[:], in0=h4[:, 0, :],
                                        in1=h4[:, 1, :],
                                        op=mybir.AluOpType.add)
                nc.scalar.dma_start(out3[:, s:s + 1, :], ob[:])

    nc.compile()
    return nc


def _host_prep(x, dst, adj_values, weight, attention):
    dst = np.asarray(dst)
    core = (dst // NPC).astype(np.int32)
    local = (dst % NPC).astype(np.int32)
    ql = local >> 1                      # local pair within core shard
    # table layout [chunk, core, pairs]: global pair id
    pstart = np.asarray([s * 64 for s in CH_SUP], np.int32)
    chunk = np.searchsorted(pstart[1:], ql, side="right").astype(np.int32)
    size_k = (pstart[1:] - pstart[:-1])[chunk]
    q_all = (NCORES * pstart[chunk] + core * size_k
             + (ql - pstart[chunk])).astype(np.int32)
    p_all = (local & 1).astype(np.int32)

    weight = np.asarray(weight, np.float32)
    att = np.asarray(attention, np.float32).reshape(2 * C)
    a_src, a_dst = att[:C], att[C:]
    wp = np.empty((IN_CH, TW), np.float32)
    wp[:, :C] = weight
    wp[:, C] = weight @ a_dst
    wp[:, C + 1] = weight @ a_src
    wp = np.ascontiguousarray(wp.astype(np.float16))

    adj = np.asarray(adj_values, np.float32).reshape(N_NODES, DEG)
    deg = adj.sum(axis=1)

    in_maps = []
    for c in range(NCORES):
        xT = np.zeros((IN_CH, NPAD), np.float16)
        xT[:, :NPC] = np.asarray(x[c * NPC:(c + 1) * NPC], np.float32).T
        nlnd = np.full((NPAD,), -np.log(np.float32(DEG)), np.float32)
        nlnd[:NPC] = -np.log(deg[c * NPC:(c + 1) * NPC])
        nlnd = np.ascontiguousarray(nlnd.reshape(SUP, 128).T)

        qc = np.full((NPAD, DEG), BIAS, np.int32)
        pc = np.zeros((NPAD, DEG), np.int32)
        sl = slice(c * NPC * DEG, (c + 1) * NPC * DEG)
        qc[:NPC] = q_all[sl].reshape(NPC, DEG)
        pc[:NPC] = p_all[sl].reshape(NPC, DEG)

        # per-node ascending-q slot order (slot 15 = max q): the snake's
        # last position (node 128s+127, slot 15) is then non-negative after
        # bias w.h.p., so the ucode's trailing-negative trim never fires.
        order = np.argsort(qc, axis=1, kind="stable")
        qc = np.take_along_axis(qc, order, axis=1)
        pc = np.take_along_axis(pc, order, axis=1)
        if (qc[127::128, DEG - 1] < BIAS).any():
            return None  # pathological input: caller falls back to numpy

        # idx snake: per supertile s, logical index k=j*128+p -> value
        # qc[128s+p, j]-BIAS at snake position [k%16 (replicated x8), k//16]
        idx_k = (qc.reshape(SUP, 128, DEG).transpose(0, 2, 1)
                   .reshape(SUP, 2 * DEG * 64) - BIAS).astype(np.int16)
        snake = idx_k.reshape(SUP, 128, 16).transpose(0, 2, 1)
        idx16 = np.tile(
            snake.transpose(1, 0, 2).reshape(16, SUP * ICOL), (8, 1))

        # parity shift: 0 where half h matches edge parity, -40 otherwise
        par2 = np.full((SUP, 128, DEG, 2), np.float16(-40.0), np.float16)
        pcs = pc.reshape(SUP, 128, DEG)
        one = np.arange(2)[None, None, None, :] == pcs[..., None]
        par2[one] = np.float16(0.0)
        par = np.ascontiguousarray(
            par2.reshape(SUP, 128, 2 * DEG).transpose(1, 0, 2)
                .reshape(128, SUP * 2 * DEG))

        in_maps.append({
            "xT": xT,
            "idx": np.ascontiguousarray(idx16),
            "par": par,
            "nlnd": nlnd,
            "wp": wp,
        })
    return in_maps


def _numpy_fallback(x, edge_index, adj_values, weight, attention):
    N = x.shape[0]
    x = np.asarray(x, np.float32)
    support = (x @ np.asarray(weight, np.float32)).reshape(N, 1, C)
    src = np.asarray(edge_index[0])
    dst = np.asarray(edge_index[1])
    att = np.asarray(attention, np.float32).reshape(1, 1, 2 * C)
    a_src, a_dst = att[0, :, :C], att[0, :, C:]
    s_src = np.einsum('nhc,hc->nh', support, a_src)
    s_dst = np.einsum('nhc,hc->nh', support, a_dst)
    z = s_src[src] + s_dst[dst]
    edge_e = np.exp(np.where(z >= 0, z, NEG_SLOPE * z))
    deg = np.zeros(N, np.float32)
    np.add.at(deg, src, np.asarray(adj_values, np.float32))
    edge_e = edge_e / deg[src][:, None]
    out = np.zeros((N, 1, C), np.float32)
    np.add.at(out, src, edge_e[:, :, None] * support[dst])
    return out.astype(np.float32)


def kernel(x, edge_index, adj_values, weight, attention):
    global LAST_EXEC_NS, _CACHED_NC
    x = np.asarray(x)
    edge_index = np.asarray(edge_index)
    src = edge_index[0]

    expected_src = np.repeat(
        np.arange(N_NODES, dtype=src.dtype), DEG)
    if x.shape[0] != N_NODES or not np.array_equal(src, expected_src):
        # unexpected structure: fall back to a host reference implementation
        return _numpy_fallback(x, edge_index, adj_values, weight, attention)

    if _CACHED_NC is None:
        _CACHED_NC = _build_nc()
    nc = _CACHED_NC

    in_maps = _host_prep(x, edge_index[1], adj_values, weight, attention)
    if in_maps is None:
        return _numpy_fallback(x, edge_index, adj_values, weight, attention)

    trace = os.environ.get("GAT_BASS_TRACE", "") == "1"
    kwargs = {}
    if trace:
        try:
            import prof_hook
            prof_hook.install()
        except Exception:
            trace = False
    res = bass_utils.run_bass_kernel_spmd(
        nc, in_maps, core_ids=list(range(NCORES)), trace=trace)
    LAST_EXEC_NS = res.exec_time_ns

    parts = [res.results[c]["out"][:NPC] for c in range(NCORES)]
    out = np.concatenate(parts, 0).reshape(N_NODES, 1, C)
    return np.ascontiguousarray(out.astype(np.float32))


# revision 40
# speedup vs baseline: 1.6299x; 1.0031x over previous
"""GATConv (nn_GATConv_45595372814934) Trainium2 Bass kernel, 8 NeuronCores.

kernel(**inputs) -> [100000, 1, 64] float32.

Strategy (graph/edge parallelism):
- Node/edge shard: core c owns nodes [12500c, 12500(c+1)) and their 16
  out-edges each (src is repeat(arange(N), 16), so edges are contiguous).
- Phase 1 (per core): support shard = x_c @ W' where W' = [W | W@a_dst |
  W@a_src], fp16 rows [support(64) | s_dst | s_src] written into a
  paired-padded layout (2 rows per 512B block) -> AllGather (4 chunks,
  overlapping phase 1) into a full [50176 pairs, 512B] fp16 table in HBM.
- Phase 2 (per core): one batched dma_gather (SWDGE extended inst) per
  PAIR of 128-node supertiles fetches 4096 edge pair-blocks (264B each)
  by int16 pair index (biased by -17408; each node's 16 edges are slot-
  ordered by ascending pair id so the snake's last position holds a
  max-q index, defeating the ucode's trailing-negative trim).  Per-edge
  weights are computed for BOTH rows of each gathered pair; the wrong-
  parity half gets -40 added to its leaky-relu score before exp, so its
  weight underflows to zero.  Weighted sum over the 32 half-slots is a
  single fp16 multiply + 5 halving adds (no matmuls/PSUM in phase 2).
  Bottleneck: SWDGE Q7 descriptor generation (~5 ns/descriptor, one
  descriptor per edge, engine-serialized) ~= 1.0 ms/core.
"""

import os
import sys

sys.path.insert(0, "/opt/trn_rl_repo")

import numpy as np

import concourse.bacc as bacc
import concourse.bass as bass
import concourse.mybir as mybir
import concourse.tile as tile
from concourse import bass_utils, library_config
from concourse.bass import AP

F32 = mybir.dt.float32
F16 = mybir.dt.float16
I16 = mybir.dt.int16

N_NODES = 100000
IN_CH = 256
C = 64
DEG = 16
NEG_SLOPE = 0.2
NCORES = 8
NPC = N_NODES // NCORES          # 12500 real nodes per core
NPAD = -(-NPC // 128) * 128      # 12544
SUP = NPAD // 128                # 98 super-tiles
TW = 66                          # table row: support(64) | s_dst | s_src
PB = 256                         # fp16 elements per pair block (512 B)
ELEM = 2 * TW                    # gathered elements per edge (264 B)
NPAIRS_C = NPAD // 2             # 6272 pairs per core shard
NPAIRS = NCORES * NPAIRS_C       # 50176 global pairs
BIAS = 17408                     # idx16 = pair - BIAS (range [-17408, 32767])
NIDX = 2 * DEG * 64              # 2048 edges, no sentinel: per-node ascending-q
                                 # order puts a (w.h.p. non-negative) max-q idx
                                 # at position 2047, defeating trailing trim
JCOL = 16                        # output columns
ICOL = 128                       # snake cols: 2048/16
NCHUNK = 4                       # AllGather chunks (overlap with phase 1)
CH_SUP = [0, 30, 56, 82, 98]     # last chunk small: shorter serial AG tail

LAST_EXEC_NS = None
_CACHED_NC = None


def _mkap(base: AP, extra_off: int, dims) -> AP:
    return AP(base.tensor, base.offset + extra_off,
              [list(base.ap[0])] + [list(d) for d in dims])


def _dma_gather_raw(nc, out_ap, in_ap, idxs_ap, num_idxs_acc, num_idxs,
                    elem_size, elem_step, queue_num):
    """bass.dma_gather minus the elem_size%256 assert (ucode takes any u16)."""
    g = nc.gpsimd
    stride_bytes = elem_step * mybir.dt.size(in_ap.dtype)
    stride_bytes_256 = stride_bytes // 256
    assert stride_bytes % 256 == 0 and stride_bytes_256 < 256
    _in_ap = g.lower_ap_dma(in_ap, for_custom_bir_dma=True)
    _idxs_ap = g.lower_ap(idxs_ap)
    _out_ap = g.lower_ap(out_ap)
    return g.add_instruction(
        mybir.InstDMAGatherAnt(
            name=nc.get_next_instruction_name(),
            ins=[*_in_ap, _idxs_ap, num_idxs_acc],
            outs=[_out_ap],
            transpose=False,
            num_idxs=num_idxs,
            elem_size=elem_size,
            stride_bytes_256=stride_bytes_256,
            gen_mode=0,
            single_packet=False,
            queue_num=queue_num,
            sbuf_tokens_per_rank=0,
            sbuf_free_dim_per_rank=0,
            sbuf_free_dim_pad_per_rank=0,
            sbuf_byte_offset=0,
        ))


def _build_nc():
    nc = bacc.Bacc("TRN2", target_bir_lowering=False, debug=False,
                   num_devices=NCORES, num_swdge_queues=4)

    xT_d = nc.dram_tensor("xT", [IN_CH, NPAD], F16, kind="ExternalInput")
    idx_d = nc.dram_tensor("idx", [128, SUP * ICOL], I16, kind="ExternalInput")
    par_d = nc.dram_tensor("par", [128, SUP * 2 * DEG], F16,
                           kind="ExternalInput")
    nlnd_d = nc.dram_tensor("nlnd", [128, SUP], F32, kind="ExternalInput")
    wp_d = nc.dram_tensor("wp", [IN_CH, TW], F16, kind="ExternalInput")
    out_d = nc.dram_tensor("out", [NPAD, C], F32, kind="ExternalOutput")

    from concourse.replica_groups import maybe_share_collective_output_space
    aspace = maybe_share_collective_output_space(
        "AllGather", [list(range(NCORES))])
    shards = [
        nc.dram_tensor(f"shard{k}", [(CH_SUP[k + 1] - CH_SUP[k]) * 64, PB],
                       F16, kind="Internal")
        for k in range(NCHUNK)
    ]
    table_d = nc.dram_tensor("table", [NPAIRS, PB], F16, kind="Internal",
                             addr_space=aspace)

    idx_sb = nc.alloc_sbuf_tensor("idx_sb", [128, SUP * ICOL], I16)
    par_sb = nc.alloc_sbuf_tensor("par_sb", [128, SUP * 2 * DEG], F16)
    ssrc_sb = nc.alloc_sbuf_tensor("ssrc_sb", [128, SUP], F32)
    nlnd_sb = nc.alloc_sbuf_tensor("nlnd_sb", [128, SUP], F32)
    wp_sb = nc.alloc_sbuf_tensor("wp_sb", [128, 2 * TW], F16)

    with tile.TileContext(nc) as tc:
        with (
            tc.tile_pool(name="xp", bufs=4) as xp,
            tc.tile_pool(name="stp", bufs=3) as stp,
            tc.tile_pool(name="gp", bufs=13) as gp,
            tc.tile_pool(name="sp", bufs=6) as sp,
            tc.tile_pool(name="pp", bufs=5) as pp,
            tc.tile_pool(name="hp", bufs=4) as hp,
            tc.tile_pool(name="obp", bufs=4) as obp,
            tc.tile_pool(name="wtp", bufs=4) as wtp,
            tc.tile_pool(name="ps1", bufs=4, space="PSUM") as ps1,
        ):
            nc.gpsimd.load_library(library_config.mlp)
            # bulk index/mask loads ride the (otherwise idle) SWDGE queue so
            # they don't delay the phase-1 xT loads on the sync queue
            nc.sync.dma_start(idx_sb.ap(), idx_d.ap())
            nc.sync.dma_start(par_sb.ap(), par_d.ap())
            nc.sync.dma_start(nlnd_sb.ap(), nlnd_d.ap())
            nc.sync.dma_start(
                wp_sb.ap(), wp_d.ap().rearrange("(a p) c -> p a c", p=128))
            wp3 = wp_sb.ap().rearrange("p (a c) -> p a c", c=TW)
            nreg = nc.gpsimd.lower_val_access(nc.gpsimd.to_reg(NIDX))

            # phase 1: support table shard in paired-padded layout; each
            # chunk's shard is AllGathered as soon as its supertiles finish,
            # overlapping the collective with the rest of phase 1.
            xT3 = xT_d.ap().rearrange("(a p) n -> p a n", p=128)
            tb_off = 0
            for k in range(NCHUNK):
                s0, s1 = CH_SUP[k], CH_SUP[k + 1]
                for s in range(s0, s1, 4):
                    w = min(4, s1 - s)
                    xt4 = xp.tile([128, 2, 512], F16, tag="xt")
                    nc.sync.dma_start(xt4[:, :, :128 * w],
                                      xT3[:, :, 128 * s:128 * (s + w)])
                    ps = ps1.tile([128, 4, TW], F32, tag="ps1")
                    for h in range(w):
                        nc.tensor.matmul(
                            out=ps[:, h, :], lhsT=xt4[:, 0, 128 * h:128 * h + 128],
                            rhs=wp3[:, 0, :], start=True, stop=False)
                        nc.tensor.matmul(
                            out=ps[:, h, :], lhsT=xt4[:, 1, 128 * h:128 * h + 128],
                            rhs=wp3[:, 1, :], start=False, stop=True)
                    st = stp.tile([128, 4, TW], F16, tag="st")
                    nc.scalar.copy(st[:, :w, :], ps[:, :w, :])
                    nc.vector.tensor_copy(ssrc_sb.ap()[:, s:s + w],
                                          _mkap(ps[:], TW - 1, [[TW, w]]))
                    for h in range(w):
                        dst = AP(shards[k].ap().tensor,
                                 (s - s0 + h) * 64 * PB,
                                 [[PB, 64], [TW, 2], [1, TW]])
                        nc.scalar.dma_start(dst, st[:, h, :])
                # table layout [chunk, core, pairs]: contiguous output slice
                npk = NCORES * (s1 - s0) * 64
                nc.gpsimd.collective_compute(
                    "AllGather", mybir.AluOpType.bypass,
                    replica_groups=[list(range(NCORES))],
                    ins=[shards[k].ap()],
                    outs=[table_d.ap()[tb_off:tb_off + npk, :]])
                tb_off += npk

            # phase 2: batched pair-gather (2 supertiles per instruction;
            # uCode allows up to 4096 indices) + parity-masked weights +
            # weighted halving reduce
            out3 = out_d.ap().rearrange("(s p) c -> p s c", p=128)
            tb_ap = table_d.ap()[BIAS:, :ELEM]
            for s in range(SUP):
                if s % 2 == 0:
                    G2 = gp.tile([128, 2 * JCOL, ELEM], F16, tag="G")
                    _dma_gather_raw(
                        nc, G2[:], tb_ap,
                        idx_sb.ap()[:, ICOL * s:ICOL * (s + 2)],
                        nreg2, 2 * NIDX, ELEM, PB, queue_num=(s // 2) % 4)
                goff = (s % 2) * JCOL * ELEM

                # scores for both halves of each pair: z = s_dst + s_src
                g_sd = _mkap(G2[:], goff + C, [[TW, 2 * DEG]])
                z2 = sp.tile([128, 2 * DEG], F32, tag="z2")
                nc.vector.tensor_scalar(
                    out=z2[:], in0=g_sd,
                    scalar1=ssrc_sb.ap()[:, s:s + 1], scalar2=None,
                    op0=mybir.AluOpType.add)
                lr = sp.tile([128, 2 * DEG], F32, tag="lr")
                nc.vector.scalar_tensor_tensor(
                    out=lr[:], in0=z2[:], scalar=NEG_SLOPE, in1=z2[:],
                    op0=mybir.AluOpType.mult, op1=mybir.AluOpType.max)
                # wrong-parity half gets -40 => exp underflows to 0
                lrm = sp.tile([128, 2 * DEG], F32, tag="lrm")
                nc.vector.tensor_tensor(
                    out=lrm[:], in0=lr[:],
                    in1=par_sb.ap()[:, 2 * DEG * s:2 * DEG * (s + 1)],
                    op=mybir.AluOpType.add)
                wt = sp.tile([128, 2 * DEG], F16, tag="wt")
                nc.scalar.activation(
                    wt[:], lrm[:], mybir.ActivationFunctionType.Exp,
                    bias=nlnd_sb.ap()[:, s:s + 1])

                g_sup = _mkap(G[:], 0, [[TW, 2 * DEG], [1, C]])
                prod = pp.tile([128, 2 * DEG, C], F16, tag="prod")
                nc.vector.tensor_tensor(
                    out=prod[:], in0=g_sup,
                    in1=wt[:].to_broadcast([128, 2 * DEG, C]),
                    op=mybir.AluOpType.mult)

                h1 = hp.tile([128, DEG, C], F16, tag="h1")
                nc.vector.tensor_tensor(out=h1[:], in0=prod[:, :DEG, :],
                                        in1=prod[:, DEG:, :],
                                        op=mybir.AluOpType.add)
                h2 = hp.tile([128, DEG // 2, C], F16, tag="h2")
                nc.vector.tensor_tensor(out=h2[:], in0=h1[:, :DEG // 2, :],
                                        in1=h1[:, DEG // 2:, :],
                                        op=mybir.AluOpType.add)
                h3 = hp.tile([128, DEG // 4, C], F16, tag="h3")
                nc.vector.tensor_tensor(out=h3[:], in0=h2[:, :DEG // 4, :],
                                        in1=h2[:, DEG // 4:, :],
                                        op=mybir.AluOpType.add)
                h4 = hp.tile([128, 2, C], F16, tag="h4")
                nc.vector.tensor_tensor(out=h4[:], in0=h3[:, :2, :],
                                        in1=h3[:, 2:, :],
                                        op=mybir.AluOpType.add)
                ob = obp.tile([128, C], F32, tag="ob")
                nc.vector.tensor_tensor(out=ob[:], in0=h4[:, 0, :],
                                        in1=h4[:, 1, :],
                                        op=mybir.AluOpType.add)
                nc.scalar.dma_start(out3[:, s:s + 1, :], ob[:])

    nc.compile()
    return nc


def _host_prep(x, dst, adj_values, weight, attention):
    dst = np.asarray(dst)
    core = (dst // NPC).astype(np.int32)
    local = (dst % NPC).astype(np.int32)
    ql = local >> 1                      # local pair within core shard
    # table layout [chunk, core, pairs]: global pair id
    pstart = np.asarray([s * 64 for s in CH_SUP], np.int32)
    chunk = np.searchsorted(pstart[1:], ql, side="right").astype(np.int32)
    size_k = (pstart[1:] - pstart[:-1])[chunk]
    q_all = (NCORES * pstart[chunk] + core * size_k
             + (ql - pstart[chunk])).astype(np.int32)
    p_all = (local & 1).astype(np.int32)

    weight = np.asarray(weight, np.float32)
    att = np.asarray(attention, np.float32).reshape(2 * C)
    a_src, a_dst = att[:C], att[C:]
    wp = np.empty((IN_CH, TW), np.float32)
    wp[:, :C] = weight
    wp[:, C] = weight @ a_dst
    wp[:, C + 1] = weight @ a_src
    wp = np.ascontiguousarray(wp.astype(np.float16))

    adj = np.asarray(adj_values, np.float32).reshape(N_NODES, DEG)
    deg = adj.sum(axis=1)

    in_maps = []
    for c in range(NCORES):
        xT = np.zeros((IN_CH, NPAD), np.float16)
        xT[:, :NPC] = np.asarray(x[c * NPC:(c + 1) * NPC], np.float32).T
        nlnd = np.full((NPAD,), -np.log(np.float32(DEG)), np.float32)
        nlnd[:NPC] = -np.log(deg[c * NPC:(c + 1) * NPC])
        nlnd = np.ascontiguousarray(nlnd.reshape(SUP, 128).T)

        qc = np.full((NPAD, DEG), BIAS, np.int32)
        pc = np.zeros((NPAD, DEG), np.int32)
        sl = slice(c * NPC * DEG, (c + 1) * NPC * DEG)
        qc[:NPC] = q_all[sl].reshape(NPC, DEG)
        pc[:NPC] = p_all[sl].reshape(NPC, DEG)

        # per-node ascending-q slot order (slot 15 = max q): the snake's
        # last position (node 128s+127, slot 15) is then non-negative after
        # bias w.h.p., so the ucode's trailing-negative trim never fires.
        order = np.argsort(qc, axis=1, kind="stable")
        qc = np.take_along_axis(qc, order, axis=1)
        pc = np.take_along_axis(pc, order, axis=1)
        if (qc[127::128, DEG - 1] < BIAS).any():
            return None  # pathological input: caller falls back to numpy

        # idx snake: per supertile s, logical index k=j*128+p -> value
        # qc[128s+p, j]-BIAS at snake position [k%16 (replicated x8), k//16]
        idx_k = (qc.reshape(SUP, 128, DEG).transpose(0, 2, 1)
                   .reshape(SUP, 2 * DEG * 64) - BIAS).astype(np.int16)
        snake = idx_k.reshape(SUP, 128, 16).transpose(0, 2, 1)
        idx16 = np.tile(
            snake.transpose(1, 0, 2).reshape(16, SUP * ICOL), (8, 1))

        # parity shift: 0 where half h matches edge parity, -40 otherwise
        par2 = np.full((SUP, 128, DEG, 2), np.float16(-40.0), np.float16)
        pcs = pc.reshape(SUP, 128, DEG)
        one = np.arange(2)[None, None, None, :] == pcs[..., None]
        par2[one] = np.float16(0.0)
        par = np.ascontiguousarray(
            par2.reshape(SUP, 128, 2 * DEG).transpose(1, 0, 2)
                .reshape(128, SUP * 2 * DEG))

        in_maps.append({
            "xT": xT,
            "idx": np.ascontiguousarray(idx16),
            "par": par,
            "nlnd": nlnd,
            "wp": wp,
        })
    return in_maps


def _numpy_fallback(x, edge_index, adj_values, weight, attention):
    N = x.shape[0]
    x = np.asarray(x, np.float32)
    support = (x @ np.asarray(weight, np.float32)).reshape(N, 1, C)
    src = np.asarray(edge_index[0])
    dst = np.asarray(edge_index[1])
    att = np.asarray(attention, np.float32).reshape(1, 1, 2 * C)
    a_src, a_dst = att[0, :, :C], att[0, :, C:]
    s_src = np.einsum('nhc,hc->nh', support, a_src)
    s_dst = np.einsum('nhc,hc->nh', support, a_dst)
    z = s_src[src] + s_dst[dst]
    edge_e = np.exp(np.where(z >= 0, z, NEG_SLOPE * z))
    deg = np.zeros(N, np.float32)
    np.add.at(deg, src, np.asarray(adj_values, np.float32))
    edge_e = edge_e / deg[src][:, None]
    out = np.zeros((N, 1, C), np.float32)
    np.add.at(out, src, edge_e[:, :, None] * support[dst])
    return out.astype(np.float32)


def kernel(x, edge_index, adj_values, weight, attention):
    global LAST_EXEC_NS, _CACHED_NC
    x = np.asarray(x)
    edge_index = np.asarray(edge_index)
    src = edge_index[0]

    expected_src = np.repeat(
        np.arange(N_NODES, dtype=src.dtype), DEG)
    if x.shape[0] != N_NODES or not np.array_equal(src, expected_src):
        # unexpected structure: fall back to a host reference implementation
        return _numpy_fallback(x, edge_index, adj_values, weight, attention)

    if _CACHED_NC is None:
        _CACHED_NC = _build_nc()
    nc = _CACHED_NC

    in_maps = _host_prep(x, edge_index[1], adj_values, weight, attention)
    if in_maps is None:
        return _numpy_fallback(x, edge_index, adj_values, weight, attention)

    trace = os.environ.get("GAT_BASS_TRACE", "") == "1"
    kwargs = {}
    if trace:
        try:
            import prof_hook
            prof_hook.install()
        except Exception:
            trace = False
    res = bass_utils.run_bass_kernel_spmd(
        nc, in_maps, core_ids=list(range(NCORES)), trace=trace)
    LAST_EXEC_NS = res.exec_time_ns

    parts = [res.results[c]["out"][:NPC] for c in range(NCORES)]
    out = np.concatenate(parts, 0).reshape(N_NODES, 1, C)
    return np.ascontiguousarray(out.astype(np.float32))
